# revision 1
# baseline (speedup 1.0000x reference)
"""MoE layer (top-2 of 8 experts, d_model=1024, d_hidden=512) on 8 trn2 cores.

Token-parallel: each core processes 1024 of the 8192 tokens against all 8
experts. Gating (logits, top-2, softmax) is computed on-device in fp32;
the two expert MLP matmuls run in fp32r (full PE speed). The gate weight is
folded into the combine step as a per-partition scalar multiply-accumulate,
so non-selected experts contribute 0 exactly as in the reference math.

Layout notes:
  - x arrives host-transposed per-shard as xT [D, TC] so both MLP matmuls can
    contract over the partition dimension with weights in native layout.
  - mm1 produces hT [C, tokens] (expert weights stationary), mm2 flips back to
    token-major y [tokens, D] (hT chunks stationary) so the gate is a
    per-partition [128,1] scalar and the output DMAs out in native layout.
"""

import os
import sys

import numpy as np

for _p in ("/opt/trn_rl_repo", "/root/.axon_site/_ro/trn_rl_repo"):
    if _p not in sys.path and os.path.isdir(_p):
        sys.path.append(_p)

P = 128
D_MODEL = 1024
C_HID = 512
N_EXP = 8
TOP_K = 2
N_CORES = 8
T_FULL = 4 * 2048
TC = T_FULL // N_CORES  # tokens per core

KC = D_MODEL // P  # 8 contraction chunks over D
CC = C_HID // P    # 4 contraction chunks over C
TT = TC // P       # 8 token chunks of 128
NT = 512           # moving-dim chunk (tokens) for mm1
DH = 512           # moving-dim chunk (d_model) for mm2

_CACHE = {}

# set by test harness to capture profiling info
TRACE = False
LAST_RESULT = None


def _install_ntff_hook_shim():
    """Register the axon NTFF profile hook if the image's antenv lacks it.

    bass_utils resolves the hook via `antenv.axon_hooks`; when that module is
    absent, tracing silently degrades. The hook implementation itself ships
    with the axon boot package, so wire it up through sys.modules.
    """
    try:
        from antenv.axon_hooks import get_axon_ntff_profile_hook  # noqa: F401
        return  # real module present
    except ImportError:
        pass
    try:
        import types

        if "/root/.axon_site" not in sys.path and os.path.isdir("/root/.axon_site"):
            sys.path.append("/root/.axon_site")
        from trn_agent_boot.trn_boot import _ntff_profile_via_ctypes

        so_path = "/opt/axon/libaxon_pjrt.so"
        if not os.path.exists(so_path):
            return
        hook = _ntff_profile_via_ctypes(so_path)
        mod = types.ModuleType("antenv.axon_hooks")
        mod.get_axon_ntff_profile_hook = lambda: hook
        mod.set_axon_ntff_profile_hook = lambda h: None
        import antenv

        antenv.axon_hooks = mod
        sys.modules["antenv.axon_hooks"] = mod
    except Exception:
        pass


def _split_excess_waits(nc, mybir, maxw=1):
    """This walrus build accepts at most one semaphore wait per instruction.

    Tile emits instructions (notably the kernel-tail drain) with several
    waits; split the extras into preceding single-wait NoOps on the same
    engine — program order makes the chain equivalent.
    """
    for f in nc.m.functions:
        for bb in f.blocks:
            out = []
            changed = False
            for ins in bb.instructions:
                si = ins.sync_info
                waits = list(si.on_wait) if (si is not None and si.on_wait) else []
                if len(waits) > maxw:
                    extra, keep = waits[:-maxw], waits[-maxw:]
                    for ci in range(0, len(extra), maxw):
                        out.append(mybir.InstNoOp(
                            name=f"{ins.name}_ws{ci}",
                            sync_info=mybir.SyncInfo(
                                on_wait=list(extra[ci:ci + maxw]), on_update=[]
                            ),
                            engine=ins.engine,
                            bass_nofuse=True,
                        ))
                    si.on_wait = keep
                    changed = True
                out.append(ins)
            if changed:
                bb.instructions = out


def _build_nc():
    import concourse.bass as bass
    import concourse.mybir as mybir
    import concourse.tile as tile
    from contextlib import ExitStack

    dt = mybir.dt
    f32 = dt.float32
    f32r = dt.float32r
    f16 = dt.float16
    AX = mybir.AxisListType
    OP = mybir.AluOpType
    ACT = mybir.ActivationFunctionType

    nc = bass.Bass("TRN2", debug=False)

    xT = nc.dram_tensor("xT", [D_MODEL, TC], f16, kind="ExternalInput")
    dxT = nc.dram_tensor("dxT", [D_MODEL, TC], f16, kind="ExternalInput")
    wgp = nc.dram_tensor("wgp", [D_MODEL, 2 * N_EXP], f16, kind="ExternalInput")
    w1 = nc.dram_tensor("w1", [N_EXP, D_MODEL, C_HID], f16, kind="ExternalInput")
    w2 = nc.dram_tensor("w2", [N_EXP, C_HID, D_MODEL], f16, kind="ExternalInput")
    id8 = nc.dram_tensor("id8", [2 * N_EXP, 2 * N_EXP], f32, kind="ExternalInput")
    out = nc.dram_tensor("out", [TC, D_MODEL], f32, kind="ExternalOutput")

    with tile.TileContext(nc) as tc:
        with ExitStack() as ctx:
            cpool = ctx.enter_context(tc.tile_pool(name="cpool", bufs=1))
            wpool = ctx.enter_context(tc.tile_pool(name="wpool", bufs=2))
            hpool = ctx.enter_context(tc.tile_pool(name="hpool", bufs=2))
            gpool = ctx.enter_context(tc.tile_pool(name="gpool", bufs=2))
            psum_mm = ctx.enter_context(tc.tile_pool(name="psum_mm", bufs=4, space="PSUM"))
            psum_sm = ctx.enter_context(tc.tile_pool(name="psum_sm", bufs=3, space="PSUM"))

            xt_sb = cpool.tile([P, KC, TC], f16, name="xt_sb")
            dxt_sb = cpool.tile([P, KC, TC], f16, name="dxt_sb")
            wg_sb = cpool.tile([P, KC, 2 * N_EXP], f16, name="wg_sb")
            out_sb = cpool.tile([P, TT, D_MODEL], f32, name="out_sb")
            gate_sb = cpool.tile([P, TT, N_EXP], f32, name="gate_sb")
            id16_sb = cpool.tile([2 * N_EXP, 2 * N_EXP], f32, name="id16_sb")
            lgT_sb = cpool.tile([P, 2, TC], f32, name="lgT_sb")

            # DMA order tuned for earliest PE start: expert-0 weights and the
            # fp16 activations feed mm1(e0); the fp32 gating inputs follow in
            # small chunks so logits stream in behind it.
            w1_sb0 = wpool.tile([P, KC, C_HID], f16, name="w1_sb", tag="w1")
            w1r0 = w1[0].rearrange("(kc p) c -> p kc c", p=P)
            nc.sync.dma_start(w1_sb0[:, :, 0:P], w1r0[:, :, 0:P])
            nc.sync.dma_start(
                xt_sb[:, :, 0:NT],
                xT[:, 0:NT].rearrange("(kc p) t -> p kc t", p=P))
            for q in range(1, CC):
                nc.sync.dma_start(
                    w1_sb0[:, :, q * P:(q + 1) * P], w1r0[:, :, q * P:(q + 1) * P])
            nc.sync.dma_start(
                xt_sb[:, :, NT:TC],
                xT[:, NT:TC].rearrange("(kc p) t -> p kc t", p=P))
            w2_sb0 = wpool.tile([P, CC, D_MODEL], f16, name="w2_sb", tag="w2")
            nc.sync.dma_start(
                w2_sb0[:], w2[0].rearrange("(cc p) d -> p cc d", p=P))
            nc.sync.dma_start(wg_sb[:], wgp[:].rearrange("(kc p) e -> p kc e", p=P))
            nc.sync.dma_start(id16_sb[:], id8[:])
            for th2 in range(2):
                sl = slice(th2 * NT, (th2 + 1) * NT)
                nc.sync.dma_start(
                    dxt_sb[:, :, sl],
                    dxT[:, sl].rearrange("(kc p) t -> p kc t", p=P))

            def emit_mm1(w1_sb):
                ht_sb = hpool.tile([P, CC, TC], f16, name="ht_sb", tag="ht")
                for th in range(TC // NT):
                    for cm in range(CC):
                        ps_h = psum_mm.tile([P, NT], f32, name="ps_h", tag="ps")
                        for kc in range(KC):
                            nc.tensor.matmul(
                                ps_h[:],
                                lhsT=w1_sb[:, kc, cm * P:(cm + 1) * P],
                                rhs=xt_sb[:, kc, th * NT:(th + 1) * NT],
                                start=(kc == 0),
                                stop=(kc == KC - 1),
                            )
                        nc.scalar.activation(
                            ht_sb[:, cm, th * NT:(th + 1) * NT], ps_h[:], ACT.Relu
                        )
                return ht_sb

            def emit_mm2(e, w2_sb, ht_sb):
                for tt in range(TT):
                    for dh in range(D_MODEL // DH):
                        ps_y = psum_mm.tile([P, DH], f32, name="ps_y", tag="ps")
                        for cc in range(CC):
                            nc.tensor.matmul(
                                ps_y[:],
                                lhsT=ht_sb[:, cc, tt * P:(tt + 1) * P],
                                rhs=w2_sb[:, cc, dh * DH:(dh + 1) * DH],
                                start=(cc == 0),
                                stop=(cc == CC - 1),
                            )
                        o_sl = out_sb[:, tt, dh * DH:(dh + 1) * DH]
                        g_col = gate_sb[:, tt, e:e + 1]
                        if e == 0:
                            nc.vector.tensor_single_scalar(
                                o_sl, ps_y[:], g_col, op=OP.mult
                            )
                        else:
                            nc.vector.scalar_tensor_tensor(
                                o_sl, in0=ps_y[:], scalar=g_col, in1=o_sl,
                                op0=OP.mult, op1=OP.add,
                            )

            # expert-0 mm1 first in the PE stream (its inputs land first)
            ht_sb0 = emit_mm1(w1_sb0)

            # ---- routing: logitsT = [wg16|dwg].T @ x16 (+ dx correction into
            # rows 0:8), transposed back per chunk; top-2/softmax batched
            # across all 8 token chunks.
            logits_all = cpool.tile([P, TT, N_EXP], f32, name="logits_all")
            for th in range(2):
                ps_lt = psum_mm.tile([P, NT], f32, name="ps_lt", tag="ps")
                for kc in range(KC):
                    nc.tensor.matmul(
                        ps_lt[0:2 * N_EXP, :],
                        lhsT=wg_sb[:, kc, :],
                        rhs=xt_sb[:, kc, th * NT:(th + 1) * NT],
                        start=(kc == 0),
                        stop=False,
                    )
                for kc in range(KC):
                    nc.tensor.matmul(
                        ps_lt[0:N_EXP, :],
                        lhsT=wg_sb[:, kc, 0:N_EXP],
                        rhs=dxt_sb[:, kc, th * NT:(th + 1) * NT],
                        start=False,
                        stop=(kc == KC - 1),
                    )
                nc.vector.tensor_copy(
                    lgT_sb[0:2 * N_EXP, 0, th * NT:(th + 1) * NT],
                    ps_lt[0:2 * N_EXP, :])
            for tt in range(TT):
                ps_l = psum_sm.tile([P, 2 * N_EXP], f32, name="ps_l", tag="ps_l")
                nc.tensor.transpose(
                    ps_l[:], lgT_sb[0:2 * N_EXP, 0, tt * P:(tt + 1) * P], id16_sb[:])
                lgh = gpool.tile([P, N_EXP], f32, name="lgh", tag="lgh")
                nc.vector.tensor_copy(lgh[:], ps_l[:, N_EXP:2 * N_EXP])
                nc.vector.tensor_add(
                    logits_all[:, tt, :], ps_l[:, 0:N_EXP], lgh[:])

            def b3(ap2d):
                return ap2d.rearrange("p (t o) -> p t o", o=1).to_broadcast(
                    [P, TT, N_EXP])

            m1a = gpool.tile([P, TT], f32, name="m1a", tag="m1a", bufs=1)
            nc.vector.reduce_max(m1a[:], logits_all[:], axis=AX.X)
            eq1a = gpool.tile([P, TT, N_EXP], f32, name="eq1a", tag="eq1a", bufs=1)
            nc.vector.tensor_tensor(
                eq1a[:], logits_all[:], b3(m1a[:]), op=OP.is_equal)
            mska = gpool.tile([P, TT, N_EXP], f32, name="mska", tag="mska", bufs=1)
            nc.vector.scalar_tensor_tensor(
                mska[:], in0=eq1a[:], scalar=-1e30, in1=logits_all[:],
                op0=OP.mult, op1=OP.add)
            m2a = gpool.tile([P, TT], f32, name="m2a", tag="m2a", bufs=1)
            nc.vector.reduce_max(m2a[:], mska[:], axis=AX.X)
            eq2a = gpool.tile([P, TT, N_EXP], f32, name="eq2a", tag="eq2a", bufs=1)
            nc.vector.tensor_tensor(
                eq2a[:], mska[:], b3(m2a[:]), op=OP.is_equal)
            dlta = gpool.tile([P, TT], f32, name="dlta", tag="dlta", bufs=1)
            nc.vector.tensor_tensor(dlta[:], m2a[:], m1a[:], op=OP.subtract)
            p2a = gpool.tile([P, TT], f32, name="p2a", tag="p2a", bufs=1)
            nc.scalar.activation(p2a[:], dlta[:], ACT.Sigmoid)
            p1a = gpool.tile([P, TT], f32, name="p1a", tag="p1a", bufs=1)
            nc.vector.tensor_scalar(
                p1a[:], p2a[:], -1.0, 1.0, op0=OP.mult, op1=OP.add)
            g1a = gpool.tile([P, TT, N_EXP], f32, name="g1a", tag="g1a", bufs=1)
            nc.vector.tensor_mul(g1a[:], eq1a[:], b3(p1a[:]))
            nc.vector.tensor_mul(eq2a[:], eq2a[:], b3(p2a[:]))
            nc.vector.tensor_add(gate_sb[:], g1a[:], eq2a[:])

            # ---- experts, software-pipelined: mm1(e+1) is emitted between
            # the gating block and mm2(e) so the gate-chain latency hides
            # behind independent matmul work.
            ht_cur, w2_cur = ht_sb0, w2_sb0
            for e in range(N_EXP):
                if e + 1 < N_EXP:
                    w1_sb = wpool.tile([P, KC, C_HID], f16, name="w1_sb", tag="w1")
                    nc.sync.dma_start(
                        w1_sb[:], w1[e + 1].rearrange("(kc p) c -> p kc c", p=P)
                    )
                    w2_nxt = wpool.tile([P, CC, D_MODEL], f16, name="w2_sb", tag="w2")
                    nc.sync.dma_start(
                        w2_nxt[:], w2[e + 1].rearrange("(cc p) d -> p cc d", p=P)
                    )
                    ht_nxt = emit_mm1(w1_sb)
                else:
                    ht_nxt = w2_nxt = None
                emit_mm2(e, w2_cur, ht_cur)
                ht_cur, w2_cur = ht_nxt, w2_nxt

            for tt in range(TT):
                nc.sync.dma_start(
                    out[tt * P:(tt + 1) * P, :], out_sb[:, tt, :])

    _split_excess_waits(nc, mybir)
    return nc


def _get_nc():
    if "nc" not in _CACHE:
        _CACHE["nc"] = _build_nc()
    return _CACHE["nc"]


def kernel(**inputs) -> np.ndarray:
    global LAST_RESULT
    x = np.ascontiguousarray(np.asarray(inputs["x"], dtype=np.float32))
    Wg = np.ascontiguousarray(np.asarray(inputs["Wg"], dtype=np.float32))
    W1 = np.ascontiguousarray(np.asarray(inputs["W1"], dtype=np.float32))
    W2 = np.ascontiguousarray(np.asarray(inputs["W2"], dtype=np.float32))

    B, S, D = x.shape
    xf = x.reshape(B * S, D)
    w1h = np.ascontiguousarray(W1.astype(np.float16))
    w2h = np.ascontiguousarray(W2.astype(np.float16))
    wg16c = Wg.astype(np.float16)
    dwgc = (Wg - wg16c.astype(np.float32)).astype(np.float16)
    wgpc = np.ascontiguousarray(np.concatenate([wg16c, dwgc], axis=1))
    in_maps = []
    for i in range(N_CORES):
        shard = xf[i * TC:(i + 1) * TC]
        xt = np.ascontiguousarray(shard.T)
        xt16 = np.ascontiguousarray(xt.astype(np.float16))
        in_maps.append({
            "xT": xt16,
            "dxT": np.ascontiguousarray(
                (xt - xt16.astype(np.float32)).astype(np.float16)),
            "wgp": wgpc,
            "id8": np.eye(2 * N_EXP, dtype=np.float32),
            "w1": w1h,
            "w2": w2h,
        })

    from concourse.bass_utils import run_bass_kernel_spmd

    _install_ntff_hook_shim()
    nc = _get_nc()
    res = run_bass_kernel_spmd(
        nc, in_maps, core_ids=list(range(N_CORES)), trace=TRACE
    )
    LAST_RESULT = res
    out = np.concatenate([r["out"] for r in res.results], axis=0)
    return out.reshape(B, S, D)



# revision 2
# speedup vs baseline: 2.9861x; 2.9861x over previous
"""MoE layer (top-2 of 8 experts, d_model=1024, d_hidden=512) on 8 trn2 cores.

Expert-parallel sparse dispatch: routing (gating logits, top-2, softmax) runs
on the host in fp32 as part of the dispatch step; only the tokens actually
routed to an expert are sent to that expert's core, so each core does ~1/4 of
the dense per-expert FLOPs the reference materializes.

Each core processes a fixed capacity of tokens split into a main segment
(its own expert, main_t tiles of 128) and one overflow tile that can carry
another expert's excess (keeps the SPMD program uniform while balancing
load: per-core work = max 17 tiles = 2176 tokens instead of padding every
core to the largest expert count). Gate weights are applied on-device as a
per-partition scalar multiply; the host scatter-adds the two expert
contributions per token (indices within one expert segment are unique, so
fancy-index += is exact).

Layout notes (same as the dense baseline):
  - x arrives host-gathered and transposed per-core as xT [D, cap] fp16 so
    both MLP matmuls contract over the partition dimension.
  - mm1 produces hT [C, tokens] (expert W1 stationary), mm2 flips back to
    token-major y [tokens, D] (hT chunks stationary) so the gate is a
    per-partition [128,1] scalar and the output DMAs out in native layout.
"""

import os
import sys

import numpy as np

for _p in ("/opt/trn_rl_repo", "/root/.axon_site/_ro/trn_rl_repo"):
    if _p not in sys.path and os.path.isdir(_p):
        sys.path.append(_p)

P = 128
D_MODEL = 1024
C_HID = 512
N_EXP = 8
N_CORES = 8

KC = D_MODEL // P  # 8 contraction chunks over D
CC = C_HID // P    # 4 contraction chunks over C
NT = 512           # moving-dim chunk (tokens) for mm1
DH = 512           # moving-dim chunk (d_model) for mm2

_CACHE = {}

# set by test harness to capture profiling info
TRACE = False
LAST_RESULT = None


def _install_ntff_hook_shim():
    """Register the axon NTFF profile hook if the image's antenv lacks it.

    bass_utils resolves the hook via `antenv.axon_hooks`; when that module is
    absent, tracing silently degrades. The hook implementation itself ships
    with the axon boot package, so wire it up through sys.modules.
    """
    try:
        from antenv.axon_hooks import get_axon_ntff_profile_hook  # noqa: F401
        return  # real module present
    except ImportError:
        pass
    try:
        import types

        if "/root/.axon_site" not in sys.path and os.path.isdir("/root/.axon_site"):
            sys.path.append("/root/.axon_site")
        from trn_agent_boot.trn_boot import _ntff_profile_via_ctypes

        so_path = "/opt/axon/libaxon_pjrt.so"
        if not os.path.exists(so_path):
            return
        hook = _ntff_profile_via_ctypes(so_path)
        mod = types.ModuleType("antenv.axon_hooks")
        mod.get_axon_ntff_profile_hook = lambda: hook
        mod.set_axon_ntff_profile_hook = lambda h: None
        import antenv

        antenv.axon_hooks = mod
        sys.modules["antenv.axon_hooks"] = mod
    except Exception:
        pass


def _split_excess_waits(nc, mybir, maxw=1):
    """This walrus build accepts at most one semaphore wait per instruction.

    Tile emits instructions (notably the kernel-tail drain) with several
    waits; split the extras into preceding single-wait NoOps on the same
    engine — program order makes the chain equivalent.
    """
    for f in nc.m.functions:
        for bb in f.blocks:
            out = []
            changed = False
            for ins in bb.instructions:
                si = ins.sync_info
                waits = list(si.on_wait) if (si is not None and si.on_wait) else []
                if len(waits) > maxw:
                    extra, keep = waits[:-maxw], waits[-maxw:]
                    for ci in range(0, len(extra), maxw):
                        out.append(mybir.InstNoOp(
                            name=f"{ins.name}_ws{ci}",
                            sync_info=mybir.SyncInfo(
                                on_wait=list(extra[ci:ci + maxw]), on_update=[]
                            ),
                            engine=ins.engine,
                            bass_nofuse=True,
                        ))
                    si.on_wait = keep
                    changed = True
                out.append(ins)
            if changed:
                bb.instructions = out
    return nc


def _build_nc(main_t, ov_t):
    import concourse.bass as bass
    import concourse.mybir as mybir
    import concourse.tile as tile
    from contextlib import ExitStack

    dt = mybir.dt
    f32 = dt.float32
    f16 = dt.float16
    OP = mybir.AluOpType
    ACT = mybir.ActivationFunctionType

    capt = main_t + ov_t
    cap = capt * P
    main_cap = main_t * P
    assert main_cap % NT == 0 and 0 < ov_t * P <= NT

    nc = bass.Bass("TRN2", debug=False)

    xT = nc.dram_tensor("xT", [D_MODEL, cap], f16, kind="ExternalInput")
    w1a = nc.dram_tensor("w1a", [D_MODEL, C_HID], f16, kind="ExternalInput")
    w2a = nc.dram_tensor("w2a", [C_HID, D_MODEL], f16, kind="ExternalInput")
    w1b = nc.dram_tensor("w1b", [D_MODEL, C_HID], f16, kind="ExternalInput")
    w2b = nc.dram_tensor("w2b", [C_HID, D_MODEL], f16, kind="ExternalInput")
    gt = nc.dram_tensor("gt", [P, capt], f32, kind="ExternalInput")
    out = nc.dram_tensor("out", [cap, D_MODEL], f32, kind="ExternalOutput")

    with tile.TileContext(nc) as tc:
        with ExitStack() as ctx:
            cpool = ctx.enter_context(tc.tile_pool(name="cpool", bufs=1))
            opool = ctx.enter_context(tc.tile_pool(name="opool", bufs=4))
            psum_h = ctx.enter_context(
                tc.tile_pool(name="psum_h", bufs=4, space="PSUM"))
            psum_y = ctx.enter_context(
                tc.tile_pool(name="psum_y", bufs=4, space="PSUM"))

            xt_sb = cpool.tile([P, KC, cap], f16, name="xt_sb")
            ht_sb = cpool.tile([P, CC, cap], f16, name="ht_sb")
            w1a_sb = cpool.tile([P, KC, C_HID], f16, name="w1a_sb")
            w2a_sb = cpool.tile([P, CC, D_MODEL], f16, name="w2a_sb")
            w1b_sb = cpool.tile([P, KC, C_HID], f16, name="w1b_sb")
            w2b_sb = cpool.tile([P, CC, D_MODEL], f16, name="w2b_sb")
            gt_sb = cpool.tile([P, capt], f32, name="gt_sb")

            def dma_w1(sb, t, nchunk=CC):
                r = t.rearrange("(kc p) c -> p kc c", p=P)
                step = C_HID // nchunk
                for q in range(nchunk):
                    nc.sync.dma_start(
                        sb[:, :, q * step:(q + 1) * step],
                        r[:, :, q * step:(q + 1) * step])

            def dma_w2(sb, t, nchunk=2):
                r = t.rearrange("(cc p) d -> p cc d", p=P)
                step = D_MODEL // nchunk
                for q in range(nchunk):
                    nc.sync.dma_start(
                        sb[:, :, q * step:(q + 1) * step],
                        r[:, :, q * step:(q + 1) * step])

            def dma_x(s, n):
                nc.sync.dma_start(
                    xt_sb[:, :, s:s + n],
                    xT[:, s:s + n].rearrange("(kc p) t -> p kc t", p=P))

            # blocks: (token_start, n_tokens, weight_slot)
            blocks = [(i * NT, NT, 0) for i in range(main_cap // NT)]
            blocks.append((main_cap, ov_t * P, 1))

            # DMA order tuned for earliest PE start: expert weights and the
            # first x block feed mm1(b0); later blocks stream in behind.
            dma_w1(w1a_sb, w1a)
            dma_x(0, NT)
            nc.sync.dma_start(gt_sb[:], gt[:])
            dma_x(NT, NT)
            dma_w2(w2a_sb, w2a)
            for (s, n, _slot) in blocks[2:-1]:
                dma_x(s, n)
            dma_w1(w1b_sb, w1b, nchunk=1)
            dma_x(blocks[-1][0], blocks[-1][1])
            dma_w2(w2b_sb, w2b, nchunk=1)

            w1s = [w1a_sb, w1b_sb]
            w2s = [w2a_sb, w2b_sb]

            def mm1(s, n, w1_sb):
                for cm in range(CC):
                    ps = psum_h.tile([P, NT], f32, name="ps_h", tag="psh")
                    for kc in range(KC):
                        nc.tensor.matmul(
                            ps[:, 0:n],
                            lhsT=w1_sb[:, kc, cm * P:(cm + 1) * P],
                            rhs=xt_sb[:, kc, s:s + n],
                            start=(kc == 0),
                            stop=(kc == KC - 1),
                        )
                    nc.scalar.activation(
                        ht_sb[:, cm, s:s + n], ps[:, 0:n], ACT.Relu)

            def mm2(tt, w2_sb):
                o_sb = opool.tile([P, D_MODEL], f32, name="o_sb", tag="o")
                for dh in range(D_MODEL // DH):
                    ps = psum_y.tile([P, DH], f32, name="ps_y", tag="psy")
                    for cc in range(CC):
                        nc.tensor.matmul(
                            ps[:],
                            lhsT=ht_sb[:, cc, tt * P:(tt + 1) * P],
                            rhs=w2_sb[:, cc, dh * DH:(dh + 1) * DH],
                            start=(cc == 0),
                            stop=(cc == CC - 1),
                        )
                    nc.vector.tensor_single_scalar(
                        o_sb[:, dh * DH:(dh + 1) * DH], ps[:],
                        gt_sb[:, tt:tt + 1], op=OP.mult)
                nc.sync.dma_start(out[tt * P:(tt + 1) * P, :], o_sb[:])

            # software pipeline: mm1(b) then mm2 of block b-1, so relu /
            # gate-mult / out-DMA of one block hide behind the next block's
            # matmuls and output traffic spreads across the whole kernel.
            prev = None
            for (s, n, slot) in blocks:
                mm1(s, n, w1s[slot])
                if prev is not None:
                    ps_, pn_, pslot_ = prev
                    for tt in range(ps_ // P, (ps_ + pn_) // P):
                        mm2(tt, w2s[pslot_])
                prev = (s, n, slot)
            ps_, pn_, pslot_ = prev
            for tt in range(ps_ // P, (ps_ + pn_) // P):
                mm2(tt, w2s[pslot_])

    import concourse.mybir as mybir
    _split_excess_waits(nc, mybir)
    return nc


def _get_nc(main_t, ov_t):
    key = (main_t, ov_t)
    if key not in _CACHE:
        _CACHE[key] = _build_nc(main_t, ov_t)
    return _CACHE[key]


def kernel(**inputs) -> np.ndarray:
    global LAST_RESULT
    x = np.ascontiguousarray(np.asarray(inputs["x"], dtype=np.float32))
    Wg = np.ascontiguousarray(np.asarray(inputs["Wg"], dtype=np.float32))
    W1 = np.ascontiguousarray(np.asarray(inputs["W1"], dtype=np.float32))
    W2 = np.ascontiguousarray(np.asarray(inputs["W2"], dtype=np.float32))

    B, S, D = x.shape
    T = B * S
    xf = x.reshape(T, D)

    # ---- routing on host (fp32, same math as the reference gating)
    logits = xf @ Wg                       # [T, E] fp32
    r = np.arange(T)
    e1 = np.argmax(logits, axis=1)
    l2 = logits.copy()
    l2[r, e1] = -np.inf
    e2 = np.argmax(l2, axis=1)
    s1 = logits[r, e1]
    s2 = logits[r, e2]
    z = np.exp(s2 - s1)                    # s1 >= s2, so z in (0, 1]
    p1 = (1.0 / (1.0 + z)).astype(np.float32)
    p2 = (z / (1.0 + z)).astype(np.float32)

    idxs, gates = [], []
    for e in range(N_EXP):
        m1 = e1 == e
        idx = np.nonzero(m1 | (e2 == e))[0]
        idxs.append(idx)
        gates.append(np.where(m1[idx], p1[idx], p2[idx]).astype(np.float32))

    # main segment sized so every expert's excess fits in the 8 overflow
    # tiles (one 128-token tile per core)
    main_t = 16
    while sum(-(-max(0, len(ix) - main_t * P) // P) for ix in idxs) > N_CORES:
        main_t += 1
    main_cap = main_t * P
    ov_t = 1
    cap = (main_t + ov_t) * P

    # overflow chunks (expert, offset_into_idx, n), assigned one per core
    chunks = []
    for e in range(N_EXP):
        o = main_cap
        while o < len(idxs[e]):
            chunks.append((e, o, min(P, len(idxs[e]) - o)))
            o += P
    donors = list(chunks) + [None] * (N_CORES - len(chunks))

    w1h = [np.ascontiguousarray(W1[e].astype(np.float16)) for e in range(N_EXP)]
    w2h = [np.ascontiguousarray(W2[e].astype(np.float16)) for e in range(N_EXP)]
    zw1 = np.zeros((D_MODEL, C_HID), np.float16)
    zw2 = np.zeros((C_HID, D_MODEL), np.float16)

    in_maps = []
    for c in range(N_CORES):
        n_own = min(len(idxs[c]), main_cap)
        xp = np.zeros((cap, D_MODEL), np.float32)
        gp = np.zeros(cap, np.float32)
        xp[:n_own] = xf[idxs[c][:n_own]]
        gp[:n_own] = gates[c][:n_own]
        if donors[c] is not None:
            e, o, n = donors[c]
            xp[main_cap:main_cap + n] = xf[idxs[e][o:o + n]]
            gp[main_cap:main_cap + n] = gates[e][o:o + n]
        in_maps.append({
            "xT": xp.T.astype(np.float16, order="C"),
            "gt": np.ascontiguousarray(gp.reshape(main_t + ov_t, P).T),
            "w1a": w1h[c],
            "w2a": w2h[c],
            "w1b": w1h[donors[c][0]] if donors[c] is not None else zw1,
            "w2b": w2h[donors[c][0]] if donors[c] is not None else zw2,
        })

    from concourse.bass_utils import run_bass_kernel_spmd

    _install_ntff_hook_shim()
    nc = _get_nc(main_t, ov_t)
    res = run_bass_kernel_spmd(
        nc, in_maps, core_ids=list(range(N_CORES)), trace=TRACE
    )
    LAST_RESULT = res

    # ---- combine: scatter-add the (gate-scaled) expert outputs per token.
    # Indices are unique within each segment, so fancy += is exact.
    outf = np.zeros((T, D_MODEL), np.float32)
    for c in range(N_CORES):
        y = np.asarray(res.results[c]["out"], dtype=np.float32)
        n_own = min(len(idxs[c]), main_cap)
        outf[idxs[c][:n_own]] += y[:n_own]
        if donors[c] is not None:
            e, o, n = donors[c]
            outf[idxs[e][o:o + n]] += y[main_cap:main_cap + n]
    return outf.reshape(B, S, D)
